# revision 20
# baseline (speedup 1.0000x reference)
"""Distributed multi-head attention kernel for 8 TRN2 NeuronCores.

Reference computation (per batch b):
    q = x @ wq.T ; k = x @ wk.T ; v = x @ wv.T          (heads split from 512 -> 8 x 64)
    attn = softmax(q k^T / sqrt(64)) ; o = attn @ v
    y = concat_heads(o) @ wproj.T

Sharding: core c handles batch b = c // 4 and head-block hb = c % 4
(2 heads = 128 channels).  Within a 4-core replica group (one batch) the
normalized head outputs are AllGather'ed (chunked along the query axis,
overlapped with attention compute) and each core computes a column block
(128 output channels) of the final projection.

All device matmuls run in bf16 with fp32 PSUM accumulation; softmax exp
runs on the Scalar engine in fp32 (no max-subtraction needed: logits are
O(+-6)), with the denominator obtained by appending a ones-column to v.
"""

import sys

sys.path.insert(0, "/opt/trn_rl_repo")

import ml_dtypes
import numpy as np

B = 2
N = 3136
DIM = 512
HEADS = 8
HD = 64
SCALE = HD**-0.5
N_CORES = 8
GROUPS = [[0, 1, 2, 3], [4, 5, 6, 7]]

BF16 = ml_dtypes.bfloat16

# query/row chunks of 512 (last 64), key tiles of 128 (last 64)
QCH = [(o, min(512, N - o)) for o in range(0, N, 512)]
MT = [(o, min(128, N - o)) for o in range(0, N, 128)]
# AllGather parts: after query-chunk qi, gather columns [off, off+len)
AG_AFTER = {2: (0, 1536), 4: (1536, 1024), 6: (2560, 576)}

_CACHE = {}


def _build(debug_dumps=False):
    import concourse.bacc as bacc
    import concourse.mybir as mybir
    import concourse.tile as tile
    from concourse.bass_interp import get_hw_module

    F32 = mybir.dt.float32
    BF = mybir.dt.bfloat16

    nc = bacc.Bacc("TRN2", target_bir_lowering=False, debug=False, num_devices=N_CORES)

    xT_d = nc.dram_tensor("xT", [DIM, N], BF, kind="ExternalInput")
    wq_d = nc.dram_tensor("wqT", [DIM, 128], BF, kind="ExternalInput")
    wk_d = nc.dram_tensor("wkT", [DIM, 128], BF, kind="ExternalInput")
    wv_d = nc.dram_tensor("wvT", [DIM, 128], BF, kind="ExternalInput")
    wp_d = nc.dram_tensor("wpT", [DIM, 128], BF, kind="ExternalInput")
    out_d = nc.dram_tensor("out", [128, N], F32, kind="ExternalOutput")

    EXP = mybir.ActivationFunctionType.Exp

    with tile.TileContext(nc) as tc:
        with (
            tc.tile_pool(name="const", bufs=1) as cp,
            tc.tile_pool(name="big", bufs=1) as bp,
            tc.tile_pool(name="attn", bufs=3) as ap_,
            tc.tile_pool(name="norm", bufs=2) as np_,
            tc.tile_pool(name="gat", bufs=2) as gp,
            tc.tile_pool(name="psum", bufs=2, space="PSUM") as pa,
            tc.tile_pool(name="dram", bufs=1, space="DRAM") as dram,
        ):
            # ---- tiny warmup collective: absorbs collective-subsystem init
            # concurrently with the compute prologue ----
            wtiny = cp.tile([1, 16], BF)
            nc.vector.memset(wtiny[:], 0.0)
            wi = dram.tile([1, 16], BF)
            wo = dram.tile([4, 16], BF)
            nc.gpsimd.dma_start(wi[:], wtiny[:])
            nc.gpsimd.collective_compute(
                "AllGather",
                mybir.AluOpType.bypass,
                replica_groups=GROUPS,
                ins=[wi.opt()],
                outs=[wo.opt()],
            )

            # ---- load inputs (weights first: they gate the first matmuls) ----
            wqT = cp.tile([128, 4, 128], BF)
            wkT = cp.tile([128, 4, 128], BF)
            wvT = cp.tile([128, 4, 128], BF)
            wpT = cp.tile([128, 4, 128], BF)
            for t, d in ((wkT, wk_d), (wqT, wq_d), (wvT, wv_d), (wpT, wp_d)):
                for k in range(4):
                    nc.gpsimd.dma_start(t[:, k, :], d[128 * k : 128 * (k + 1), :])
            xT = bp.tile([128, 4, N], BF)  # xT[:, k, :] = channels [128k,128k+128)
            for lo, hi in ((0, 512), (512, 1792), (1792, N)):
                for k in range(4):
                    nc.sync.dma_start(
                        xT[:, k, lo:hi], xT_d[128 * k : 128 * (k + 1), lo:hi]
                    )

            # ---- qkv projections ----
            qT = bp.tile([128, N], BF)  # rows 0-63 head0, 64-127 head1
            kT = bp.tile([128, N], BF)
            v1 = bp.tile([128, len(MT), 2, HD + 1], BF)  # [key, mtile, head, hd|1]
            nc.vector.memset(v1[:, :, :, HD : HD + 1], 1.0)

            def produce_chunk(wt, dst, qo, qn):
                ps = pa.tile([128, 2, 512], F32, tag="pp", name="ps")
                for k in range(4):
                    nc.tensor.matmul(
                        ps[:, 0, :qn],
                        wt[:, k, :],
                        xT[:, k, qo : qo + qn],
                        start=(k == 0),
                        stop=(k == 3),
                    )
                nc.vector.tensor_copy(dst[:, qo : qo + qn], ps[:, 0, :qn])

            def produce_v1(mi):
                mo, mn = MT[mi]
                ps = pa.tile([128, 2, 512], F32, tag="pp", name="ps")
                for k in range(4):
                    nc.tensor.matmul(
                        ps[:mn, 0, :128],
                        xT[:, k, mo : mo + mn],
                        wvT[:, k, :],
                        start=(k == 0),
                        stop=(k == 3),
                    )
                nc.vector.tensor_copy(v1[:mn, mi, 0, 0:HD], ps[:mn, 0, 0:HD])
                nc.vector.tensor_copy(v1[:mn, mi, 1, 0:HD], ps[:mn, 0, HD:128])

            # up-front: all of kT, first two qT chunks, first three v tiles;
            # the rest is produced just-in-time inside the first two
            # attention chunks so the Scalar engine (the bottleneck) starts
            # exp'ing ~35us earlier.
            for qo, qn in QCH:
                produce_chunk(wkT, kT, qo, qn)
            for qo, qn in QCH[0:2]:
                produce_chunk(wqT, qT, qo, qn)
            for mi in range(3):
                produce_v1(mi)

            # ---- attention ----
            outn = [bp.tile([64, N], BF, name=f"outn{h}") for h in range(2)]
            ag_bufs = []  # (ago, part_offset, part_len) per part, for projection

            if True:
                for qi, (qo, qn) in enumerate(QCH):
                    po = pa.tile([128, 2, 512], F32, tag="po")
                    for mi, (mo, mn) in enumerate(MT):
                        pp = pa.tile([128, 2, 512], F32, tag="pp")
                        at = ap_.tile([128, 2, 512], BF, tag="at")
                        for h in range(2):
                            hs = slice(64 * h, 64 * (h + 1))
                            nc.tensor.matmul(
                                pp[:mn, h, :qn],
                                kT[hs, mo : mo + mn],
                                qT[hs, qo : qo + qn],
                                start=True,
                                stop=True,
                            )
                        nc.scalar.activation(at[:mn, :, :qn], pp[:mn, :, :qn], EXP)
                        for h in range(2):
                            nc.tensor.matmul(
                                po[0:65, h, :qn],
                                v1[:mn, mi, h, :],
                                at[:mn, h, :qn],
                                start=(mi == 0),
                                stop=(mi == len(MT) - 1),
                            )
                        if qi == 0 and mi + 3 < len(MT):
                            produce_v1(mi + 3)
                        elif qi in (1, 2) and mi % 10 == 0:
                            j = 2 + (qi - 1) * 3 + mi // 10
                            if j < len(QCH):
                                produce_chunk(wqT, qT, *QCH[j])
                    # normalize rows 0-63 by row 64 (softmax denominator).
                    # NB: partition_broadcast mis-reads APs whose base
                    # partition != 0 on HW, so land the reciprocal on p0.
                    for h in range(2):
                        rs = np_.tile([1, 512], F32, tag="rs")
                        nc.vector.reciprocal(rs[0:1, :qn], po[64:65, h, :qn])
                        rb = np_.tile([64, 512], F32, tag="rb")
                        nc.gpsimd.partition_broadcast(rb[:, :qn], rs[0:1, :qn])
                        nc.vector.tensor_mul(
                            outn[h][:, qo : qo + qn], po[0:64, h, :qn], rb[:, :qn]
                        )

                    if qi not in AG_AFTER:
                        continue
                    # ---- AllGather this part (overlaps remaining attention) ----
                    pof, pln = AG_AFTER[qi]
                    pi = list(AG_AFTER).index(qi)
                    agi = dram.tile([128, pln], BF, name=f"agi{pi}")
                    ago = dram.tile([DIM, pln], BF, name=f"ago{pi}")
                    for h in range(2):
                        nc.sync.dma_start(
                            agi[64 * h : 64 * (h + 1), :], outn[h][:, pof : pof + pln]
                        )
                    nc.gpsimd.collective_compute(
                        "AllGather",
                        mybir.AluOpType.bypass,
                        replica_groups=GROUPS,
                        ins=[agi.opt()],
                        outs=[ago.opt()],
                    )
                    ag_bufs.append((ago, pof, pln))

            # ---- projection (column-parallel: this core's 128 out-channels) ----
            yt = bp.tile([128, N], F32)
            if debug_dumps:
                dbg = {
                    name: nc.dram_tensor(name, shape, BF, kind="ExternalOutput")
                    for name, shape in (
                        ("dbg_qT", [128, N]),
                        ("dbg_kT", [128, N]),
                        ("dbg_outn0", [64, N]),
                        ("dbg_outn1", [64, N]),
                    )
                }
            if True:
                for ago, pof, pln in ag_bufs:
                    g = gp.tile([128, 4, 1536], BF, tag="g")
                    for k in range(4):
                        nc.sync.dma_start(
                            g[:, k, :pln], ago[128 * k : 128 * (k + 1), :]
                        )
                    for qo, qn in [(o, n) for o, n in QCH if pof <= o < pof + pln]:
                        py = pa.tile([128, 2, 512], F32, tag="pp")
                        for k in range(4):
                            nc.tensor.matmul(
                                py[:, 0, :qn],
                                wpT[:, k, :],
                                g[:, k, qo - pof : qo - pof + qn],
                                start=(k == 0),
                                stop=(k == 3),
                            )
                        nc.vector.tensor_copy(yt[:, qo : qo + qn], py[:, 0, :qn])
                    nc.sync.dma_start(out_d[:, pof : pof + pln], yt[:, pof : pof + pln])

            if debug_dumps:
                nc.sync.dma_start(dbg["dbg_qT"][:], qT[:])
                nc.sync.dma_start(dbg["dbg_kT"][:], kT[:])
                nc.sync.dma_start(dbg["dbg_outn0"][:], outn[0][:])
                nc.sync.dma_start(dbg["dbg_outn1"][:], outn[1][:])

    nc.compile()
    nc.m = get_hw_module(nc.m)
    return nc


def _shard(x, wq, wk, wv, wproj):
    x = np.asarray(x, dtype=np.float32)
    wq = np.asarray(wq, dtype=np.float32)
    wk = np.asarray(wk, dtype=np.float32)
    wv = np.asarray(wv, dtype=np.float32)
    wproj = np.asarray(wproj, dtype=np.float32)

    xT = [np.ascontiguousarray(x[b].T).astype(BF16) for b in range(B)]
    in_maps = []
    for c in range(N_CORES):
        b, hb = c // 4, c % 4
        rows = slice(128 * hb, 128 * (hb + 1))
        in_maps.append(
            {
                "xT": xT[b],
                "wqT": np.ascontiguousarray((wq[rows] * SCALE).T).astype(BF16),
                "wkT": np.ascontiguousarray(wk[rows].T).astype(BF16),
                "wvT": np.ascontiguousarray(wv[rows].T).astype(BF16),
                "wpT": np.ascontiguousarray(wproj[rows].T).astype(BF16),
            }
        )
    return in_maps


def _run(inputs, trace=False):
    from concourse.bass_utils import run_bass_kernel_spmd

    if "nc" not in _CACHE:
        _CACHE["nc"] = _build()
    nc = _CACHE["nc"]
    in_maps = _shard(**inputs)
    res = run_bass_kernel_spmd(
        nc, in_maps, core_ids=list(range(N_CORES)), trace=trace
    )
    out = np.empty((B, N, DIM), dtype=np.float32)
    for c in range(N_CORES):
        b, hb = c // 4, c % 4
        out[b, :, 128 * hb : 128 * (hb + 1)] = res.results[c]["out"].T
    return out, res.exec_time_ns


def kernel(**inputs) -> np.ndarray:
    return _run(inputs, trace=False)[0]


# revision 21
# speedup vs baseline: 1.0455x; 1.0455x over previous
"""Distributed multi-head attention kernel for 8 TRN2 NeuronCores.

Reference computation (per batch b):
    q = x @ wq.T ; k = x @ wk.T ; v = x @ wv.T          (heads split from 512 -> 8 x 64)
    attn = softmax(q k^T / sqrt(64)) ; o = attn @ v
    y = concat_heads(o) @ wproj.T

Sharding: core c handles batch b = c // 4 and head-block hb = c % 4
(2 heads = 128 channels).  Within a 4-core replica group (one batch) the
normalized head outputs are AllGather'ed (chunked along the query axis,
overlapped with attention compute) and each core computes a column block
(128 output channels) of the final projection.

All device matmuls run in bf16 with fp32 PSUM accumulation; softmax exp
runs on the Scalar engine in fp32 (no max-subtraction needed: logits are
O(+-6)), with the denominator obtained by appending a ones-column to v.
"""

import sys

sys.path.insert(0, "/opt/trn_rl_repo")

import ml_dtypes
import numpy as np

B = 2
N = 3136
DIM = 512
HEADS = 8
HD = 64
SCALE = HD**-0.5
N_CORES = 8
GROUPS = [[0, 1, 2, 3], [4, 5, 6, 7]]

BF16 = ml_dtypes.bfloat16

# query/row chunks of 512 (last 64), key tiles of 128 (last 64)
QCH = [(o, min(512, N - o)) for o in range(0, N, 512)]
MT = [(o, min(128, N - o)) for o in range(0, N, 128)]
# AllGather parts: after query-chunk qi, gather columns [off, off+len)
AG_AFTER = {2: (0, 1536), 4: (1536, 1024), 5: (2560, 512), 6: (3072, 64)}

_CACHE = {}


def _build(debug_dumps=False):
    import concourse.bacc as bacc
    import concourse.mybir as mybir
    import concourse.tile as tile
    from concourse.bass_interp import get_hw_module

    F32 = mybir.dt.float32
    BF = mybir.dt.bfloat16

    nc = bacc.Bacc("TRN2", target_bir_lowering=False, debug=False, num_devices=N_CORES)

    xT_d = nc.dram_tensor("xT", [DIM, N], BF, kind="ExternalInput")
    wq_d = nc.dram_tensor("wqT", [DIM, 128], BF, kind="ExternalInput")
    wk_d = nc.dram_tensor("wkT", [DIM, 128], BF, kind="ExternalInput")
    wv_d = nc.dram_tensor("wvT", [DIM, 128], BF, kind="ExternalInput")
    wp_d = nc.dram_tensor("wpT", [DIM, 128], BF, kind="ExternalInput")
    out_d = nc.dram_tensor("out", [128, N], F32, kind="ExternalOutput")

    EXP = mybir.ActivationFunctionType.Exp

    with tile.TileContext(nc) as tc:
        with (
            tc.tile_pool(name="const", bufs=1) as cp,
            tc.tile_pool(name="big", bufs=1) as bp,
            tc.tile_pool(name="attn", bufs=3) as ap_,
            tc.tile_pool(name="norm", bufs=2) as np_,
            tc.tile_pool(name="gat", bufs=2) as gp,
            tc.tile_pool(name="psum", bufs=2, space="PSUM") as pa,
            tc.tile_pool(name="dram", bufs=1, space="DRAM") as dram,
        ):
            # ---- tiny warmup collective: absorbs collective-subsystem init
            # concurrently with the compute prologue ----
            wtiny = cp.tile([1, 16], BF)
            nc.vector.memset(wtiny[:], 0.0)
            wi = dram.tile([1, 16], BF)
            wo = dram.tile([4, 16], BF)
            nc.gpsimd.dma_start(wi[:], wtiny[:])
            nc.gpsimd.collective_compute(
                "AllGather",
                mybir.AluOpType.bypass,
                replica_groups=GROUPS,
                ins=[wi.opt()],
                outs=[wo.opt()],
            )

            # ---- load inputs (weights first: they gate the first matmuls) ----
            wqT = cp.tile([128, 4, 128], BF)
            wkT = cp.tile([128, 4, 128], BF)
            wvT = cp.tile([128, 4, 128], BF)
            wpT = cp.tile([128, 4, 128], BF)
            for t, d in ((wkT, wk_d), (wqT, wq_d), (wvT, wv_d), (wpT, wp_d)):
                for k in range(4):
                    nc.gpsimd.dma_start(t[:, k, :], d[128 * k : 128 * (k + 1), :])
            xT = bp.tile([128, 4, N], BF)  # xT[:, k, :] = channels [128k,128k+128)
            for lo, hi in ((0, 512), (512, 1792), (1792, N)):
                for k in range(4):
                    nc.sync.dma_start(
                        xT[:, k, lo:hi], xT_d[128 * k : 128 * (k + 1), lo:hi]
                    )

            # ---- qkv projections ----
            qT = bp.tile([128, N], BF)  # rows 0-63 head0, 64-127 head1
            kT = bp.tile([128, N], BF)
            v1 = bp.tile([128, len(MT), 2, HD + 1], BF)  # [key, mtile, head, hd|1]
            nc.vector.memset(v1[:, :, :, HD : HD + 1], 1.0)

            def produce_chunk(wt, dst, qo, qn):
                ps = pa.tile([128, 2, 512], F32, tag="pp", name="ps")
                for k in range(4):
                    nc.tensor.matmul(
                        ps[:, 0, :qn],
                        wt[:, k, :],
                        xT[:, k, qo : qo + qn],
                        start=(k == 0),
                        stop=(k == 3),
                    )
                nc.vector.tensor_copy(dst[:, qo : qo + qn], ps[:, 0, :qn])

            def produce_v1(mi):
                mo, mn = MT[mi]
                ps = pa.tile([128, 2, 512], F32, tag="pp", name="ps")
                for k in range(4):
                    nc.tensor.matmul(
                        ps[:mn, 0, :128],
                        xT[:, k, mo : mo + mn],
                        wvT[:, k, :],
                        start=(k == 0),
                        stop=(k == 3),
                    )
                nc.vector.tensor_copy(v1[:mn, mi, 0, 0:HD], ps[:mn, 0, 0:HD])
                nc.vector.tensor_copy(v1[:mn, mi, 1, 0:HD], ps[:mn, 0, HD:128])

            # up-front: all of kT, first two qT chunks, first three v tiles;
            # the rest is produced just-in-time inside the first two
            # attention chunks so the Scalar engine (the bottleneck) starts
            # exp'ing ~35us earlier.
            for qo, qn in QCH:
                produce_chunk(wkT, kT, qo, qn)
            for qo, qn in QCH[0:2]:
                produce_chunk(wqT, qT, qo, qn)
            for mi in range(3):
                produce_v1(mi)

            # ---- attention ----
            outn = [bp.tile([64, N], BF, name=f"outn{h}") for h in range(2)]
            ag_bufs = []  # (ago, part_offset, part_len) per part, for projection

            if True:
                for qi, (qo, qn) in enumerate(QCH):
                    po = pa.tile([128, 2, 512], F32, tag="po")
                    for mi, (mo, mn) in enumerate(MT):
                        pp = pa.tile([128, 2, 512], F32, tag="pp")
                        at = ap_.tile([128, 2, 512], BF, tag="at")
                        for h in range(2):
                            hs = slice(64 * h, 64 * (h + 1))
                            nc.tensor.matmul(
                                pp[:mn, h, :qn],
                                kT[hs, mo : mo + mn],
                                qT[hs, qo : qo + qn],
                                start=True,
                                stop=True,
                            )
                        nc.scalar.activation(at[:mn, :, :qn], pp[:mn, :, :qn], EXP)
                        for h in range(2):
                            nc.tensor.matmul(
                                po[0:65, h, :qn],
                                v1[:mn, mi, h, :],
                                at[:mn, h, :qn],
                                start=(mi == 0),
                                stop=(mi == len(MT) - 1),
                            )
                        if qi == 0 and mi + 3 < len(MT):
                            produce_v1(mi + 3)
                        elif qi in (1, 2) and mi % 10 == 0:
                            j = 2 + (qi - 1) * 3 + mi // 10
                            if j < len(QCH):
                                produce_chunk(wqT, qT, *QCH[j])
                    # normalize rows 0-63 by row 64 (softmax denominator).
                    # NB: partition_broadcast mis-reads APs whose base
                    # partition != 0 on HW, so land the reciprocal on p0.
                    for h in range(2):
                        rs = np_.tile([1, 512], F32, tag="rs")
                        nc.vector.reciprocal(rs[0:1, :qn], po[64:65, h, :qn])
                        rb = np_.tile([64, 512], F32, tag="rb")
                        nc.gpsimd.partition_broadcast(rb[:, :qn], rs[0:1, :qn])
                        nc.vector.tensor_mul(
                            outn[h][:, qo : qo + qn], po[0:64, h, :qn], rb[:, :qn]
                        )

                    if qi not in AG_AFTER:
                        continue
                    # ---- AllGather this part (overlaps remaining attention) ----
                    pof, pln = AG_AFTER[qi]
                    pi = list(AG_AFTER).index(qi)
                    agi = dram.tile([128, pln], BF, name=f"agi{pi}")
                    ago = dram.tile([DIM, pln], BF, name=f"ago{pi}")
                    for h in range(2):
                        nc.sync.dma_start(
                            agi[64 * h : 64 * (h + 1), :], outn[h][:, pof : pof + pln]
                        )
                    nc.gpsimd.collective_compute(
                        "AllGather",
                        mybir.AluOpType.bypass,
                        replica_groups=GROUPS,
                        ins=[agi.opt()],
                        outs=[ago.opt()],
                    )
                    ag_bufs.append((ago, pof, pln))

            # ---- projection (column-parallel: this core's 128 out-channels) ----
            yt = bp.tile([128, N], F32)
            if debug_dumps:
                dbg = {
                    name: nc.dram_tensor(name, shape, BF, kind="ExternalOutput")
                    for name, shape in (
                        ("dbg_qT", [128, N]),
                        ("dbg_kT", [128, N]),
                        ("dbg_outn0", [64, N]),
                        ("dbg_outn1", [64, N]),
                    )
                }
            if True:
                for ago, pof, pln in ag_bufs:
                    g = gp.tile([128, 4, 1536], BF, tag="g")
                    for k in range(4):
                        nc.sync.dma_start(
                            g[:, k, :pln], ago[128 * k : 128 * (k + 1), :]
                        )
                    for qo, qn in [(o, n) for o, n in QCH if pof <= o < pof + pln]:
                        py = pa.tile([128, 2, 512], F32, tag="pp")
                        for k in range(4):
                            nc.tensor.matmul(
                                py[:, 0, :qn],
                                wpT[:, k, :],
                                g[:, k, qo - pof : qo - pof + qn],
                                start=(k == 0),
                                stop=(k == 3),
                            )
                        nc.vector.tensor_copy(yt[:, qo : qo + qn], py[:, 0, :qn])
                    nc.sync.dma_start(out_d[:, pof : pof + pln], yt[:, pof : pof + pln])

            if debug_dumps:
                nc.sync.dma_start(dbg["dbg_qT"][:], qT[:])
                nc.sync.dma_start(dbg["dbg_kT"][:], kT[:])
                nc.sync.dma_start(dbg["dbg_outn0"][:], outn[0][:])
                nc.sync.dma_start(dbg["dbg_outn1"][:], outn[1][:])

    nc.compile()
    nc.m = get_hw_module(nc.m)
    return nc


def _shard(x, wq, wk, wv, wproj):
    x = np.asarray(x, dtype=np.float32)
    wq = np.asarray(wq, dtype=np.float32)
    wk = np.asarray(wk, dtype=np.float32)
    wv = np.asarray(wv, dtype=np.float32)
    wproj = np.asarray(wproj, dtype=np.float32)

    xT = [np.ascontiguousarray(x[b].T).astype(BF16) for b in range(B)]
    in_maps = []
    for c in range(N_CORES):
        b, hb = c // 4, c % 4
        rows = slice(128 * hb, 128 * (hb + 1))
        in_maps.append(
            {
                "xT": xT[b],
                "wqT": np.ascontiguousarray((wq[rows] * SCALE).T).astype(BF16),
                "wkT": np.ascontiguousarray(wk[rows].T).astype(BF16),
                "wvT": np.ascontiguousarray(wv[rows].T).astype(BF16),
                "wpT": np.ascontiguousarray(wproj[rows].T).astype(BF16),
            }
        )
    return in_maps


def _run(inputs, trace=False):
    from concourse.bass_utils import run_bass_kernel_spmd

    if "nc" not in _CACHE:
        _CACHE["nc"] = _build()
    nc = _CACHE["nc"]
    in_maps = _shard(**inputs)
    res = run_bass_kernel_spmd(
        nc, in_maps, core_ids=list(range(N_CORES)), trace=trace
    )
    out = np.empty((B, N, DIM), dtype=np.float32)
    for c in range(N_CORES):
        b, hb = c // 4, c % 4
        out[b, :, 128 * hb : 128 * (hb + 1)] = res.results[c]["out"].T
    return out, res.exec_time_ns


def kernel(**inputs) -> np.ndarray:
    return _run(inputs, trace=False)[0]
